# revision 13
# baseline (speedup 1.0000x reference)
"""SE(3) attention block (GNN message passing) on 8 Trainium2 NeuronCores.

Strategy (slot format, v3.2)
----------------------------
Nodes are sorted by in-degree (host) and cut into tiles of 128 nodes.
Tiles are grouped into batches of 8*b tiles (b per core, SPMD-identical
shapes) padded to the batch max degree S; degree sorting keeps slot
padding at ~2-4% of E.

Each node-row owns its incoming edges as "slots" 0..S-1, so the segment
softmax and the weighted aggregation become *free-axis* operations on the
node-partitioned tile -- no one-hot matrices, no per-edge gathered query,
no cross-device collectives.  Per batch (b tiles, S slots):

  1. prodT[f, (t, s, n)] = kT * qT (DVE, bf16 2x; q broadcast over slots
     via a 0-stride AP dim -- queries ship once per node, not per edge)
  2. scores[n, (t, s, h)]: b*S head-mask matmuls (PE, otherwise idle)
  3. exw[n, (t, s, f)] = exp(scores / sqrt(NF)) widened 16x in one ACT op
     per tile (replicated 0-stride read straight from PSUM); a second
     tiny ACT exp writes the narrow (h, s)-major copy for ssum
  4. evex = v * exw (DVE, bf16 2x, whole batch)
  5. agg[n, (t, f)]: dense pairwise slot-fold tree (bf16 2x adds, final
     add in f32), one instruction per level per batch
  6. ssum[n, (t, h)] = contiguous reduce of the narrow ex (f32),
     inv = 1/ssum (no eps: pad-slot design keeps ssum > 0)
  7. out = agg * inv (bf16 out)

Batching equal-S tiles keeps instruction counts (and sequencer/semaphore
overhead) low.  Input DMAs issue from the GPSIMD queue (25ns/issue vs
565ns on sync).  GPSIMD compute is intentionally unused: concurrent
GPSIMD tensor ops slow DVE ops by 2.5-3x (measured SBUF contention).

Padding slots carry k_pad = -C * q_h / |q_h|^2 per head-block so every
head scores -C (ex ~ 1e-13) and v_pad = 0 -- no masks needed.
"""

import math
import numpy as np

# ---------------------------------------------------------------- constants
N_CORES = 8
P = 128                 # partitions / nodes per tile
H = 8                   # heads
NF = 128                # features per edge (32*4)
HS = NF // H            # head size (16)
INV_SQRT_NF = 1.0 / math.sqrt(NF)
C_PAD = 345.0           # pad-slot per-head score magnitude (scaled: ~-30.5)
MAX_BATCH_SLOTS = 44    # b*S cap (PSUM bank + SBUF budget)
MAX_B = 6               # tiles per core per batch cap

_CACHE = {}
LAST_RESULTS = None     # BassKernelResults of the most recent run (for test.py)


# ---------------------------------------------------------------- device IR
def build_nc(batch_prof):
    """Per-core Bass/Tile program; identical on all 8 cores (SPMD).

    batch_prof: tuple of (S, b) per batch.
    """
    from contextlib import ExitStack

    import concourse.bacc as bacc
    import concourse.mybir as mybir
    from concourse.tile import TileContext

    f32 = mybir.dt.float32
    bf16 = mybir.dt.bfloat16
    W = int(sum(S * b for S, b in batch_prof)) * P
    Wq = int(sum(b for S, b in batch_prof)) * P

    nc = bacc.Bacc("TRN2", target_bir_lowering=False, debug=False)
    kT_d = nc.dram_tensor("kT", [P, W], bf16, kind="ExternalInput")
    v_d = nc.dram_tensor("v", [P, W], bf16, kind="ExternalInput")
    qT_d = nc.dram_tensor("qT", [P, Wq], bf16, kind="ExternalInput")
    hm_d = nc.dram_tensor("hm", [P, H], bf16, kind="ExternalInput")
    id_d = nc.dram_tensor("ident", [P, P], bf16, kind="ExternalInput")
    out_d = nc.dram_tensor("out", [P, Wq], bf16, kind="ExternalOutput")

    with TileContext(nc) as tc, ExitStack() as ctx:
        singles = ctx.enter_context(tc.tile_pool(name="singles", bufs=1))
        inp = ctx.enter_context(tc.tile_pool(name="inp", bufs=3))
        mid = ctx.enter_context(tc.tile_pool(name="mid", bufs=2))
        sml = ctx.enter_context(tc.tile_pool(name="sml", bufs=3))
        ps = ctx.enter_context(tc.tile_pool(name="ps", bufs=6, space="PSUM"))

        hm = singles.tile([P, H], bf16)
        nc.sync.dma_start(out=hm[:], in_=hm_d[:, :])
        ident = singles.tile([P, P], bf16)
        nc.sync.dma_start(out=ident[:], in_=id_d[:, :])

        offs = []
        off = offq = 0
        for S, b in batch_prof:
            offs.append((off, offq))
            off += int(S) * int(b) * P
            offq += int(b) * P

        def stage_a(i):
            """DMA in, prodT, score matmuls, exps. Returns state for stage_b."""
            S, b = int(batch_prof[i][0]), int(batch_prof[i][1])
            off, offq = offs[i]
            Wt = b * S * P
            kT = inp.tile([P, Wt], bf16, tag="kT")
            nc.gpsimd.dma_start(out=kT[:], in_=kT_d[:, off:off + Wt])
            v = inp.tile([P, Wt], bf16, tag="v")
            nc.gpsimd.dma_start(out=v[:], in_=v_d[:, off:off + Wt])
            qT = sml.tile([P, b * P], bf16, tag="qT")
            nc.gpsimd.dma_start(out=qT[:], in_=qT_d[:, offq:offq + b * P])

            prodT = mid.tile([P, Wt], bf16, tag="prodT")
            nc.vector.tensor_tensor(
                out=prodT[:].rearrange("p (t s n) -> p t s n", t=b, s=S),
                in0=kT[:].rearrange("p (t s n) -> p t s n", t=b, s=S),
                in1=qT[:].rearrange("p (t n) -> p t n", t=b)
                    .unsqueeze(2).broadcast_to([P, b, S, P]),
                op=mybir.AluOpType.mult)

            sc = ps.tile([P, b * S * H], f32, tag="sc", bufs=4)
            for ts in range(b * S):
                nc.tensor.matmul(
                    out=sc[:, ts * H:(ts + 1) * H],
                    lhsT=prodT[:, ts * P:(ts + 1) * P], rhs=hm[:],
                    start=True, stop=True)

            exw = mid.tile([P, Wt], bf16, tag="exw")
            ex = sml.tile([P, b * S * H], bf16, tag="ex")
            for t in range(b):
                nc.scalar.activation(
                    out=exw[:, t * S * P:(t + 1) * S * P]
                        .rearrange("p (s h j) -> p s h j", s=S, h=H),
                    in_=sc[:, t * S * H:(t + 1) * S * H]
                        .rearrange("p (s h) -> p s h", s=S)
                        .to_broadcast([P, S, H, HS]),
                    func=mybir.ActivationFunctionType.Exp, scale=INV_SQRT_NF)
            for t in range(b):
                nc.scalar.activation(
                    out=ex[:, t * S * H:(t + 1) * S * H]
                        .rearrange("p (h s) -> p h s", h=H),
                    in_=sc[:, t * S * H:(t + 1) * S * H]
                        .rearrange("p (s h) -> p h s", s=S),
                    func=mybir.ActivationFunctionType.Exp, scale=INV_SQRT_NF)
            return (S, b, offq, v, exw, ex)

        def stage_b(state):
            """evex, fold tree, ssum/recip, normalize, DMA out."""
            S, b, offq, v, exw, ex = state
            Wt = b * S * P
            evex = mid.tile([P, Wt], bf16, tag="evex")
            nc.vector.tensor_tensor(
                out=evex[:], in0=v[:], in1=exw[:], op=mybir.AluOpType.mult)

            # identity-weight matmuls accumulate the slot sum in PSUM
            # (exact f32, frees DVE of the whole fold tree)
            agg = ps.tile([P, b * P], f32, tag="aggps", bufs=3)
            for t in range(b):
                for s in range(S):
                    nc.tensor.matmul(
                        out=agg[:, t * P:(t + 1) * P],
                        lhsT=ident[:],
                        rhs=evex[:, (t * S + s) * P:(t * S + s + 1) * P],
                        start=(s == 0), stop=(s == S - 1))

            ssum = sml.tile([P, b * H], f32, tag="ssum")
            nc.vector.tensor_reduce(
                out=ssum[:],
                in_=ex[:].rearrange("p (t h s) -> p t h s", t=b, h=H),
                axis=mybir.AxisListType.X, op=mybir.AluOpType.add)
            inv = sml.tile([P, b * H], f32, tag="inv")
            nc.vector.reciprocal(out=inv[:], in_=ssum[:])

            outb = sml.tile([P, b * P], bf16, tag="outb")
            nc.vector.tensor_tensor(
                out=outb[:].rearrange("p (t h j) -> p t h j", t=b, h=H),
                in0=agg[:].rearrange("p (t h j) -> p t h j", t=b, h=H),
                in1=inv[:].rearrange("p (t h) -> p t h", t=b)
                    .to_broadcast([P, b, H, HS]),
                op=mybir.AluOpType.mult)
            nc.sync.dma_start(out=out_d[:, offq:offq + b * P], in_=outb[:])

        # software-pipelined emission: batch i+1's front half goes ahead of
        # batch i's back half so the in-order DVE queue never stalls on ACT
        nb = len(batch_prof)
        pend = stage_a(0)
        for i in range(1, nb):
            nxt_state = stage_a(i)
            stage_b(pend)
            pend = nxt_state
        stage_b(pend)
    nc.compile()
    return nc


# ------------------------------------------------------------ host plumbing
def _plan(edge_index, n_nodes):
    """Degree-sorted batched tile plan shared by all cores."""
    dst = np.asarray(edge_index)[1].astype(np.int64).ravel()
    n_edges = dst.shape[0]
    counts = np.bincount(dst, minlength=n_nodes)
    order_e = np.argsort(dst, kind="stable")
    cum = np.zeros(n_nodes + 1, np.int64)
    cum[1:] = np.cumsum(counts)
    nperm = np.argsort(-counts, kind="stable")

    n_tiles = -(-n_nodes // P)
    deg_desc = np.zeros(n_tiles * P, np.int64)
    deg_desc[:n_nodes] = counts[nperm]

    batches = []            # (S, b, tile_start)
    t = 0
    while t < n_tiles:
        S = max(int(deg_desc[t * P]), 4)
        rem_groups = -(-(n_tiles - t) // N_CORES)
        b = max(1, min(MAX_B, MAX_BATCH_SLOTS // S, rem_groups))
        batches.append((S, b, t))
        t += N_CORES * b
    batches = batches[-1:] + batches[:-1]   # smallest batch first: fast ramp

    total_tiles = sum(N_CORES * b for S, b, _ in batches)
    rnode = np.full(total_tiles * P, -1, np.int64)
    rnode[:n_nodes] = nperm
    return dict(counts=counts, order_e=order_e, cum=cum, rnode=rnode,
                batches=batches, n_edges=n_edges, n_nodes=n_nodes)


def _prep_inputs(value, key, query_0, query_1, plan):
    import ml_dtypes
    bf16 = ml_dtypes.bfloat16

    batches = plan["batches"]
    rnode = plan["rnode"]
    counts, order_e, cum = plan["counts"], plan["order_e"], plan["cum"]
    n_edges = plan["n_edges"]
    n_nodes = plan["n_nodes"]

    key_f = np.asarray(key, dtype=np.float32).reshape(n_edges, NF)
    val_f = np.asarray(value, dtype=np.float32).reshape(n_edges, NF)
    q_cat = np.concatenate(
        [np.asarray(query_0, np.float32), np.asarray(query_1, np.float32)],
        axis=-1).reshape(n_nodes, NF)
    # pad slots must score ~-C in EVERY head (scores are per-head dots over
    # 16 features), so normalize q per head-block
    qh = q_cat.reshape(n_nodes, H, HS)
    qh2 = np.einsum("nhj,nhj->nh", qh, qh)
    kpad = (-C_PAD * qh / np.maximum(qh2, 0.1)[:, :, None]).reshape(
        n_nodes, NF)

    W = sum(S * b for S, b, _ in batches) * P
    Wq = sum(b for S, b, _ in batches) * P
    kT_all = np.empty((N_CORES, P, W), bf16)
    v_all = np.empty((N_CORES, P, W), bf16)
    qT_all = np.empty((N_CORES, P, Wq), bf16)

    off = 0
    offq = 0
    for S, b, t0 in batches:
        nb = N_CORES * b * P
        rows = rnode[t0 * P:t0 * P + nb]
        valid_r = rows >= 0
        rr = np.where(valid_r, rows, 0)
        deg = np.where(valid_r, counts[rr], 0)
        start = cum[rr]
        sl = np.arange(S)
        eix = start[:, None] + sl[None, :]
        vmask = sl[None, :] < deg[:, None]
        eid = order_e[np.clip(eix, 0, n_edges - 1)]

        kg = key_f[eid]                        # [8*b*128, S, NF]
        kp = kpad[rr] * valid_r[:, None]
        kg = np.where(vmask[..., None], kg, kp[:, None, :])
        vg = val_f[eid]
        vg[~vmask] = 0.0
        qg = q_cat[rr] * valid_r[:, None]

        # [core, t, n, S, NF]
        kg = kg.reshape(N_CORES, b, P, S, NF)
        vg = vg.reshape(N_CORES, b, P, S, NF)
        qg = qg.reshape(N_CORES, b, P, NF)
        # kT: [f, (t, s, n)]
        kT_all[:, :, off:off + b * S * P] = kg.transpose(0, 4, 1, 3, 2).reshape(
            N_CORES, NF, b * S * P).astype(bf16)
        # v: [n, (t, s, f)]
        v_all[:, :, off:off + b * S * P] = vg.transpose(0, 2, 1, 3, 4).reshape(
            N_CORES, P, b * S * NF).astype(bf16)
        # qT: [f, (t, n)]
        qT_all[:, :, offq:offq + b * P] = qg.transpose(0, 3, 1, 2).reshape(
            N_CORES, NF, b * P).astype(bf16)
        off += b * S * P
        offq += b * P

    hm = np.zeros((NF, H), np.float32)
    for h in range(H):
        hm[h * HS:(h + 1) * HS, h] = 1.0
    hm = hm.astype(bf16)
    ident = np.eye(P, dtype=np.float32).astype(bf16)

    in_maps = []
    for c in range(N_CORES):
        in_maps.append({
            "kT": kT_all[c], "v": v_all[c], "qT": qT_all[c], "hm": hm,
            "ident": ident,
        })
    return in_maps


def _assemble(results, plan):
    batches = plan["batches"]
    n_nodes = plan["n_nodes"]
    rnode = plan["rnode"]
    out = np.zeros((n_nodes, NF), np.float32)
    for c in range(N_CORES):
        arr = np.asarray(results[c]["out"], dtype=np.float32)  # [P, Wq]
        offq = 0
        for S, b, t0 in batches:
            blk = arr[:, offq:offq + b * P].reshape(P, b, NF)
            rows = rnode[(t0 + c * b) * P:(t0 + (c + 1) * b) * P].reshape(
                b, P)
            for t in range(b):
                idx = rows[t]
                m = idx >= 0
                out[idx[m]] = blk[m, t]
            offq += b * P
    return out.reshape(n_nodes, NF // 4, 4)


def _get_nc(batch_prof):
    key = tuple(batch_prof)
    if key not in _CACHE:
        _CACHE[key] = build_nc(batch_prof)
    return _CACHE[key]


def _run(inputs, trace=False, **spmd_kwargs):
    global LAST_RESULTS
    from concourse.bass_utils import run_bass_kernel_spmd

    n_nodes = np.asarray(inputs["query_0"]).shape[0]
    plan = _plan(inputs["edge_index"], n_nodes)
    batch_prof = tuple((int(S), int(b)) for S, b, _ in plan["batches"])
    nc = _get_nc(batch_prof)
    in_maps = _prep_inputs(
        inputs["value"], inputs["key"], inputs["query_0"], inputs["query_1"],
        plan)
    res = run_bass_kernel_spmd(
        nc, in_maps, list(range(N_CORES)), trace=trace, **spmd_kwargs)
    LAST_RESULTS = res
    return _assemble(res.results, plan)


def kernel(value, key, query_0, query_1, edge_index):
    return _run({
        "value": value, "key": key, "query_0": query_0,
        "query_1": query_1, "edge_index": edge_index,
    })


# revision 14
# speedup vs baseline: 1.1760x; 1.1760x over previous
"""SE(3) attention block (GNN message passing) on 8 Trainium2 NeuronCores.

Strategy (slot format, v3.2)
----------------------------
Nodes are sorted by in-degree (host) and cut into tiles of 128 nodes.
Tiles are grouped into batches of 8*b tiles (b per core, SPMD-identical
shapes) padded to the batch max degree S; degree sorting keeps slot
padding at ~2-4% of E.

Each node-row owns its incoming edges as "slots" 0..S-1, so the segment
softmax and the weighted aggregation become *free-axis* operations on the
node-partitioned tile -- no one-hot matrices, no per-edge gathered query,
no cross-device collectives.  Per batch (b tiles, S slots):

  1. prodT[f, (t, s, n)] = kT * qT (DVE, bf16 2x; q broadcast over slots
     via a 0-stride AP dim -- queries ship once per node, not per edge)
  2. scores[n, (t, s, h)]: b*S head-mask matmuls (PE, otherwise idle)
  3. exw[n, (t, s, f)] = exp(scores / sqrt(NF)) widened 16x in one ACT op
     per tile (replicated 0-stride read straight from PSUM); a second
     tiny ACT exp writes the narrow (h, s)-major copy for ssum
  4. evex = v * exw (DVE, bf16 2x, whole batch)
  5. agg[n, (t, f)]: dense pairwise slot-fold tree (bf16 2x adds, final
     add in f32), one instruction per level per batch
  6. ssum[n, (t, h)] = contiguous reduce of the narrow ex (f32),
     inv = 1/ssum (no eps: pad-slot design keeps ssum > 0)
  7. out = agg * inv (bf16 out)

Batching equal-S tiles keeps instruction counts (and sequencer/semaphore
overhead) low.  Input DMAs issue from the GPSIMD queue (25ns/issue vs
565ns on sync).  GPSIMD compute is intentionally unused: concurrent
GPSIMD tensor ops slow DVE ops by 2.5-3x (measured SBUF contention).

Padding slots carry k_pad = -C * q_h / |q_h|^2 per head-block so every
head scores -C (ex ~ 1e-13) and v_pad = 0 -- no masks needed.
"""

import math
import numpy as np

# ---------------------------------------------------------------- constants
N_CORES = 8
P = 128                 # partitions / nodes per tile
H = 8                   # heads
NF = 128                # features per edge (32*4)
HS = NF // H            # head size (16)
INV_SQRT_NF = 1.0 / math.sqrt(NF)
C_PAD = 345.0           # pad-slot per-head score magnitude (scaled: ~-30.5)
MAX_BATCH_SLOTS = 44    # b*S cap (PSUM bank + SBUF budget)
MAX_B = 6               # tiles per core per batch cap

_CACHE = {}
LAST_RESULTS = None     # BassKernelResults of the most recent run (for test.py)


# ---------------------------------------------------------------- device IR
def build_nc(batch_prof):
    """Per-core Bass/Tile program; identical on all 8 cores (SPMD).

    batch_prof: tuple of (S, b) per batch.
    """
    from contextlib import ExitStack

    import concourse.bacc as bacc
    import concourse.mybir as mybir
    from concourse.tile import TileContext

    f32 = mybir.dt.float32
    bf16 = mybir.dt.bfloat16
    W = int(sum(S * b for S, b in batch_prof)) * P
    Wq = int(sum(b for S, b in batch_prof)) * P

    nc = bacc.Bacc("TRN2", target_bir_lowering=False, debug=False)
    kT_d = nc.dram_tensor("kT", [P, W], bf16, kind="ExternalInput")
    v_d = nc.dram_tensor("v", [P, W], bf16, kind="ExternalInput")
    qT_d = nc.dram_tensor("qT", [P, Wq], bf16, kind="ExternalInput")
    hm_d = nc.dram_tensor("hm", [P, H], bf16, kind="ExternalInput")
    out_d = nc.dram_tensor("out", [P, Wq], bf16, kind="ExternalOutput")

    with TileContext(nc) as tc, ExitStack() as ctx:
        singles = ctx.enter_context(tc.tile_pool(name="singles", bufs=1))
        inp = ctx.enter_context(tc.tile_pool(name="inp", bufs=3))
        mid = ctx.enter_context(tc.tile_pool(name="mid", bufs=2))
        sml = ctx.enter_context(tc.tile_pool(name="sml", bufs=3))
        ps = ctx.enter_context(tc.tile_pool(name="ps", bufs=6, space="PSUM"))

        hm = singles.tile([P, H], bf16)
        nc.sync.dma_start(out=hm[:], in_=hm_d[:, :])

        offs = []
        off = offq = 0
        for S, b in batch_prof:
            offs.append((off, offq))
            off += int(S) * int(b) * P
            offq += int(b) * P

        def stage_a(i):
            """DMA in, prodT, score matmuls, exps. Returns state for stage_b."""
            S, b = int(batch_prof[i][0]), int(batch_prof[i][1])
            off, offq = offs[i]
            Wt = b * S * P
            kT = inp.tile([P, Wt], bf16, tag="kT")
            nc.gpsimd.dma_start(out=kT[:], in_=kT_d[:, off:off + Wt])
            v = inp.tile([P, Wt], bf16, tag="v")
            nc.gpsimd.dma_start(out=v[:], in_=v_d[:, off:off + Wt])
            qT = sml.tile([P, b * P], bf16, tag="qT")
            nc.gpsimd.dma_start(out=qT[:], in_=qT_d[:, offq:offq + b * P])

            prodT = mid.tile([P, Wt], bf16, tag="prodT")
            nc.vector.tensor_tensor(
                out=prodT[:].rearrange("p (t s n) -> p t s n", t=b, s=S),
                in0=kT[:].rearrange("p (t s n) -> p t s n", t=b, s=S),
                in1=qT[:].rearrange("p (t n) -> p t n", t=b)
                    .unsqueeze(2).broadcast_to([P, b, S, P]),
                op=mybir.AluOpType.mult)

            sc = ps.tile([P, b * S * H], f32, tag="sc")
            for ts in range(b * S):
                nc.tensor.matmul(
                    out=sc[:, ts * H:(ts + 1) * H],
                    lhsT=prodT[:, ts * P:(ts + 1) * P], rhs=hm[:],
                    start=True, stop=True)

            exw = mid.tile([P, Wt], bf16, tag="exw")
            ex = sml.tile([P, b * S * H], bf16, tag="ex")
            for t in range(b):
                nc.scalar.activation(
                    out=exw[:, t * S * P:(t + 1) * S * P]
                        .rearrange("p (s h j) -> p s h j", s=S, h=H),
                    in_=sc[:, t * S * H:(t + 1) * S * H]
                        .rearrange("p (s h) -> p s h", s=S)
                        .to_broadcast([P, S, H, HS]),
                    func=mybir.ActivationFunctionType.Exp, scale=INV_SQRT_NF)
            for t in range(b):
                nc.scalar.activation(
                    out=ex[:, t * S * H:(t + 1) * S * H]
                        .rearrange("p (h s) -> p h s", h=H),
                    in_=sc[:, t * S * H:(t + 1) * S * H]
                        .rearrange("p (s h) -> p h s", s=S),
                    func=mybir.ActivationFunctionType.Exp, scale=INV_SQRT_NF)
            return (S, b, offq, v, exw, ex)

        def stage_b(state):
            """evex, fold tree, ssum/recip, normalize, DMA out."""
            S, b, offq, v, exw, ex = state
            Wt = b * S * P
            evex = mid.tile([P, Wt], bf16, tag="evex")
            nc.vector.tensor_tensor(
                out=evex[:], in0=v[:], in1=exw[:], op=mybir.AluOpType.mult)

            cur, src, lvl = S, evex, 0
            while cur > 2:
                nxt = cur // 2
                dst = mid.tile([P, b * nxt * P], bf16, tag=f"fold{lvl}")
                sv = src[:].rearrange("p (t s n) -> p t s n", t=b, s=cur)
                dv = dst[:].rearrange("p (t s n) -> p t s n", t=b, s=nxt)
                nc.vector.tensor_tensor(
                    out=dv, in0=sv[:, :, 0:nxt, :],
                    in1=sv[:, :, nxt:2 * nxt, :], op=mybir.AluOpType.add)
                if cur - 2 * nxt:   # odd: carry last slot into slot 0
                    nc.vector.tensor_tensor(
                        out=dv[:, :, 0:1, :], in0=dv[:, :, 0:1, :],
                        in1=sv[:, :, 2 * nxt:2 * nxt + 1, :],
                        op=mybir.AluOpType.add)
                cur, src, lvl = nxt, dst, lvl + 1
            agg = sml.tile([P, b * P], bf16, tag="agg")
            sv = src[:].rearrange("p (t s n) -> p t s n", t=b, s=cur)
            av = agg[:].rearrange("p (t n) -> p t n", t=b).unsqueeze(2)
            if cur == 2:
                nc.vector.tensor_tensor(
                    out=av, in0=sv[:, :, 0:1, :], in1=sv[:, :, 1:2, :],
                    op=mybir.AluOpType.add)
            else:
                nc.vector.tensor_scalar(
                    out=av, in0=sv[:, :, 0:1, :], scalar1=0.0, scalar2=None,
                    op0=mybir.AluOpType.add)

            ssum = sml.tile([P, b * H], f32, tag="ssum")
            nc.vector.tensor_reduce(
                out=ssum[:],
                in_=ex[:].rearrange("p (t h s) -> p t h s", t=b, h=H),
                axis=mybir.AxisListType.X, op=mybir.AluOpType.add)
            inv = sml.tile([P, b * H], f32, tag="inv")
            nc.vector.reciprocal(out=inv[:], in_=ssum[:])

            outb = sml.tile([P, b * P], bf16, tag="outb")
            nc.vector.tensor_tensor(
                out=outb[:].rearrange("p (t h j) -> p t h j", t=b, h=H),
                in0=agg[:].rearrange("p (t h j) -> p t h j", t=b, h=H),
                in1=inv[:].rearrange("p (t h) -> p t h", t=b)
                    .to_broadcast([P, b, H, HS]),
                op=mybir.AluOpType.mult)
            nc.sync.dma_start(out=out_d[:, offq:offq + b * P], in_=outb[:])

        # software-pipelined emission: batch i+1's front half goes ahead of
        # batch i's back half so the in-order DVE queue never stalls on ACT
        nb = len(batch_prof)
        pend = stage_a(0)
        for i in range(1, nb):
            nxt_state = stage_a(i)
            stage_b(pend)
            pend = nxt_state
        stage_b(pend)
    nc.compile()
    return nc


# ------------------------------------------------------------ host plumbing
def _plan(edge_index, n_nodes):
    """Degree-sorted batched tile plan shared by all cores."""
    dst = np.asarray(edge_index)[1].astype(np.int64).ravel()
    n_edges = dst.shape[0]
    counts = np.bincount(dst, minlength=n_nodes)
    order_e = np.argsort(dst, kind="stable")
    cum = np.zeros(n_nodes + 1, np.int64)
    cum[1:] = np.cumsum(counts)
    nperm = np.argsort(-counts, kind="stable")

    n_tiles = -(-n_nodes // P)
    deg_desc = np.zeros(n_tiles * P, np.int64)
    deg_desc[:n_nodes] = counts[nperm]

    batches = []            # (S, b, tile_start)
    t = 0
    while t < n_tiles:
        S = max(int(deg_desc[t * P]), 4)
        rem_groups = -(-(n_tiles - t) // N_CORES)
        b = max(1, min(MAX_B, MAX_BATCH_SLOTS // S, rem_groups))
        batches.append((S, b, t))
        t += N_CORES * b
    batches = batches[-1:] + batches[:-1]   # smallest batch first: fast ramp

    total_tiles = sum(N_CORES * b for S, b, _ in batches)
    rnode = np.full(total_tiles * P, -1, np.int64)
    rnode[:n_nodes] = nperm
    return dict(counts=counts, order_e=order_e, cum=cum, rnode=rnode,
                batches=batches, n_edges=n_edges, n_nodes=n_nodes)


def _prep_inputs(value, key, query_0, query_1, plan):
    import ml_dtypes
    bf16 = ml_dtypes.bfloat16

    batches = plan["batches"]
    rnode = plan["rnode"]
    counts, order_e, cum = plan["counts"], plan["order_e"], plan["cum"]
    n_edges = plan["n_edges"]
    n_nodes = plan["n_nodes"]

    key_f = np.asarray(key, dtype=np.float32).reshape(n_edges, NF)
    val_f = np.asarray(value, dtype=np.float32).reshape(n_edges, NF)
    q_cat = np.concatenate(
        [np.asarray(query_0, np.float32), np.asarray(query_1, np.float32)],
        axis=-1).reshape(n_nodes, NF)
    # pad slots must score ~-C in EVERY head (scores are per-head dots over
    # 16 features), so normalize q per head-block
    qh = q_cat.reshape(n_nodes, H, HS)
    qh2 = np.einsum("nhj,nhj->nh", qh, qh)
    kpad = (-C_PAD * qh / np.maximum(qh2, 0.1)[:, :, None]).reshape(
        n_nodes, NF)

    W = sum(S * b for S, b, _ in batches) * P
    Wq = sum(b for S, b, _ in batches) * P
    kT_all = np.empty((N_CORES, P, W), bf16)
    v_all = np.empty((N_CORES, P, W), bf16)
    qT_all = np.empty((N_CORES, P, Wq), bf16)

    off = 0
    offq = 0
    for S, b, t0 in batches:
        nb = N_CORES * b * P
        rows = rnode[t0 * P:t0 * P + nb]
        valid_r = rows >= 0
        rr = np.where(valid_r, rows, 0)
        deg = np.where(valid_r, counts[rr], 0)
        start = cum[rr]
        sl = np.arange(S)
        eix = start[:, None] + sl[None, :]
        vmask = sl[None, :] < deg[:, None]
        eid = order_e[np.clip(eix, 0, n_edges - 1)]

        kg = key_f[eid]                        # [8*b*128, S, NF]
        kp = kpad[rr] * valid_r[:, None]
        kg = np.where(vmask[..., None], kg, kp[:, None, :])
        vg = val_f[eid]
        vg[~vmask] = 0.0
        qg = q_cat[rr] * valid_r[:, None]

        # [core, t, n, S, NF]
        kg = kg.reshape(N_CORES, b, P, S, NF)
        vg = vg.reshape(N_CORES, b, P, S, NF)
        qg = qg.reshape(N_CORES, b, P, NF)
        # kT: [f, (t, s, n)]
        kT_all[:, :, off:off + b * S * P] = kg.transpose(0, 4, 1, 3, 2).reshape(
            N_CORES, NF, b * S * P).astype(bf16)
        # v: [n, (t, s, f)]
        v_all[:, :, off:off + b * S * P] = vg.transpose(0, 2, 1, 3, 4).reshape(
            N_CORES, P, b * S * NF).astype(bf16)
        # qT: [f, (t, n)]
        qT_all[:, :, offq:offq + b * P] = qg.transpose(0, 3, 1, 2).reshape(
            N_CORES, NF, b * P).astype(bf16)
        off += b * S * P
        offq += b * P

    hm = np.zeros((NF, H), np.float32)
    for h in range(H):
        hm[h * HS:(h + 1) * HS, h] = 1.0
    hm = hm.astype(bf16)

    in_maps = []
    for c in range(N_CORES):
        in_maps.append({
            "kT": kT_all[c], "v": v_all[c], "qT": qT_all[c], "hm": hm,
        })
    return in_maps


def _assemble(results, plan):
    batches = plan["batches"]
    n_nodes = plan["n_nodes"]
    rnode = plan["rnode"]
    out = np.zeros((n_nodes, NF), np.float32)
    for c in range(N_CORES):
        arr = np.asarray(results[c]["out"], dtype=np.float32)  # [P, Wq]
        offq = 0
        for S, b, t0 in batches:
            blk = arr[:, offq:offq + b * P].reshape(P, b, NF)
            rows = rnode[(t0 + c * b) * P:(t0 + (c + 1) * b) * P].reshape(
                b, P)
            for t in range(b):
                idx = rows[t]
                m = idx >= 0
                out[idx[m]] = blk[m, t]
            offq += b * P
    return out.reshape(n_nodes, NF // 4, 4)


def _get_nc(batch_prof):
    key = tuple(batch_prof)
    if key not in _CACHE:
        _CACHE[key] = build_nc(batch_prof)
    return _CACHE[key]


def _run(inputs, trace=False, **spmd_kwargs):
    global LAST_RESULTS
    from concourse.bass_utils import run_bass_kernel_spmd

    n_nodes = np.asarray(inputs["query_0"]).shape[0]
    plan = _plan(inputs["edge_index"], n_nodes)
    batch_prof = tuple((int(S), int(b)) for S, b, _ in plan["batches"])
    nc = _get_nc(batch_prof)
    in_maps = _prep_inputs(
        inputs["value"], inputs["key"], inputs["query_0"], inputs["query_1"],
        plan)
    res = run_bass_kernel_spmd(
        nc, in_maps, list(range(N_CORES)), trace=trace, **spmd_kwargs)
    LAST_RESULTS = res
    return _assemble(res.results, plan)


def kernel(value, key, query_0, query_1, edge_index):
    return _run({
        "value": value, "key": key, "query_0": query_0,
        "query_1": query_1, "edge_index": edge_index,
    })
